# revision 1
# baseline (speedup 1.0000x reference)
"""GatedGraphConvolution Trainium2 kernel.

out = input + segment_sum(sigmoid(g) * e, edge_sources)
  where [g|e] = input[edge_targets] @ W.T

Key algebraic fact: the per-edge message depends ONLY on the target node:
  msg_e = M[target_e],  M[n] = sigmoid(x_n @ Wg.T) * (x_n @ We.T)
so we (phase A) compute the dense M table [N, F] once per core, and
(phase B) gather M rows per edge + scatter-add by source via one-hot
matmuls accumulated in PSUM.

Sharding: nodes are sharded by SOURCE across the 8 cores (6250 nodes each);
each core receives exactly the edges whose source is in its slice, so there
are no collectives.  Edges are sorted by 128-node source "window"; each
window's edges accumulate into one PSUM tile via lhsT=onehot matmuls.
The dma_gather int16 index limit (<=32767) is handled by splitting each
window's edges into low-target (< 32768) and high-target tiles and issuing
gathers against two base offsets of the M table.
"""

import math
import os
import sys
from dataclasses import dataclass, field

import numpy as np

if "/opt/trn_rl_repo" not in sys.path:
    sys.path.insert(0, "/opt/trn_rl_repo")

import ml_dtypes

P = 128  # partitions / tile edge
F = 128  # feature dim (OUT_F == IN_F == 128)
TF = 2 * F

BF16 = ml_dtypes.bfloat16


@dataclass
class Cfg:
    n_nodes: int = 50000
    n_cores: int = 8
    half: int = 32768  # int16 index limit boundary (multiple of 128)
    gw: int = 5  # windows per gather group
    ca: int = 16  # node-tiles per phase-A chunk

    @property
    def na(self) -> int:  # node tiles in M table
        return math.ceil(self.n_nodes / P)

    @property
    def npc(self) -> int:  # nodes per core
        assert self.n_nodes % self.n_cores == 0
        return self.n_nodes // self.n_cores

    @property
    def nwin(self) -> int:  # source windows per core
        return math.ceil(self.npc / P)


@dataclass
class Plan:
    """Static schedule shared by all cores + per-core host arrays."""

    T_lo: list  # tiles per (window, low-half), max over cores
    T_hi: list
    groups: list = field(default_factory=list)  # (ws, lo_tiles, hi_tiles)
    tiles_of: dict = field(default_factory=dict)  # (w, half) -> [tile ids]
    win_tiles: dict = field(default_factory=dict)  # w -> [(tile, half, pos_in_buf)]
    TT: int = 0
    # per-core packed arrays
    srel: list = field(default_factory=list)  # [P, TT] bf16
    gidx: list = field(default_factory=list)  # [P, 8*TT] int16


def _make_schedule(cfg: Cfg, T_lo, T_hi) -> Plan:
    plan = Plan(T_lo=T_lo, T_hi=T_hi)
    t = 0
    for g0 in range(0, cfg.nwin, cfg.gw):
        ws = list(range(g0, min(g0 + cfg.gw, cfg.nwin)))
        lo, hi = [], []
        for w in ws:
            for _ in range(T_lo[w]):
                plan.tiles_of.setdefault((w, 0), []).append(t)
                lo.append((w, t))
                t += 1
        for w in ws:
            for _ in range(T_hi[w]):
                plan.tiles_of.setdefault((w, 1), []).append(t)
                hi.append((w, t))
                t += 1
        plan.groups.append((ws, lo, hi))
    plan.TT = t
    # per window: list of (global tile id, half, position within the group's
    # lo/hi gather buffer) in mm2 consumption order
    for ws, lo, hi in plan.groups:
        for w in ws:
            lst = []
            for pos, (w2, t2) in enumerate(lo):
                if w2 == w:
                    lst.append((t2, 0, pos))
            for pos, (w2, t2) in enumerate(hi):
                if w2 == w:
                    lst.append((t2, 1, pos))
            plan.win_tiles[w] = lst
    return plan


def _plan(cfg: Cfg, edge_sources: np.ndarray, edge_targets: np.ndarray) -> Plan:
    src = edge_sources.astype(np.int64)
    tgt = edge_targets.astype(np.int64)
    npc, nwin = cfg.npc, cfg.nwin

    # bucket edges per (core, window, half)
    core = src // npc
    w_all = (src % npc) // P
    srel_all = (src % npc) % P
    hi_all = (tgt >= cfg.half).astype(np.int64)

    counts = np.zeros((cfg.n_cores, nwin, 2), np.int64)
    np.add.at(counts, (core, w_all, hi_all), 1)
    tmax = counts.max(axis=0)  # [nwin, 2]
    T_lo = [int(math.ceil(tmax[w, 0] / P)) for w in range(nwin)]
    T_hi = [int(math.ceil(tmax[w, 1] / P)) for w in range(nwin)]

    plan = _make_schedule(cfg, T_lo, T_hi)

    # pack per-core slot arrays
    order = np.lexsort((hi_all, w_all, core))
    src_s, w_s, srel_s, hi_s, tgt_s = (
        src[order],
        w_all[order],
        srel_all[order],
        hi_all[order],
        tgt[order],
    )
    bounds = {}
    keys = core[order] * (nwin * 2) + w_s * 2 + hi_s
    uniq, starts = np.unique(keys, return_index=True)
    starts = list(starts) + [len(keys)]
    for i, k in enumerate(uniq):
        bounds[int(k)] = (starts[i], starts[i + 1])

    for c in range(cfg.n_cores):
        srel_arr = np.full((plan.TT * P,), 255.0, np.float32)
        gidx_arr = np.zeros((plan.TT * P,), np.int16)
        for w in range(nwin):
            for h in (0, 1):
                k = c * (nwin * 2) + w * 2 + h
                if k not in bounds:
                    continue
                a, b = bounds[k]
                tiles = plan.tiles_of.get((w, h), [])
                assert (b - a) <= len(tiles) * P
                sr = srel_s[a:b]
                tg = tgt_s[a:b] - (cfg.half if h else 0)
                for i in range(b - a):
                    t = tiles[i // P]
                    j = i % P
                    s = t * P + j
                    srel_arr[s] = sr[i]
                    gidx_arr[s] = tg[i]
        srel_host = srel_arr.reshape(plan.TT, P).T.astype(BF16)  # [P, TT]
        g16 = gidx_arr.reshape(plan.TT * 8, 16).T  # [16, TT*8]
        gidx_host = np.tile(g16, (8, 1)).astype(np.int16)  # [P, TT*8]
        plan.srel.append(np.ascontiguousarray(srel_host))
        plan.gidx.append(np.ascontiguousarray(gidx_host))
    return plan


def _host_arrays(cfg: Cfg, inp: np.ndarray, W: np.ndarray):
    """Replicated input arrays: xT (transposed node features), wT, iota."""
    n = cfg.n_nodes
    xT = np.zeros((P, cfg.na * P), np.float32)
    xT[:, :n] = inp.T
    wT = np.ascontiguousarray(W.T)  # [F, 2F]
    iota = np.tile(np.arange(P, dtype=np.float32), (P, 1))
    return (
        np.ascontiguousarray(xT.astype(BF16)),
        np.ascontiguousarray(wT.astype(BF16)),
        np.ascontiguousarray(iota.astype(BF16)),
    )


def _xs_core(cfg: Cfg, inp: np.ndarray, c: int) -> np.ndarray:
    """Per-core input slice packed [P, nwin*F]: xs[p, w*F+f] = x[c*npc+w*P+p, f]."""
    npc, nwin = cfg.npc, cfg.nwin
    sl = np.zeros((nwin * P, F), np.float32)
    sl[:npc] = inp[c * npc : (c + 1) * npc]
    return np.ascontiguousarray(sl.reshape(nwin, P, F).transpose(1, 0, 2).reshape(P, nwin * F))


def _build(cfg: Cfg, plan: Plan, enable_asserts: bool = False):
    import concourse.bacc as bacc
    import concourse.tile as tile
    from concourse import mybir

    nc = bacc.Bacc(
        "TRN2",
        target_bir_lowering=False,
        debug=False,
        enable_asserts=enable_asserts,
        num_devices=cfg.n_cores,
    )
    dt = mybir.dt

    na, nwin, npc = cfg.na, cfg.nwin, cfg.npc
    TT = plan.TT

    xT_d = nc.dram_tensor("xT", [P, na * P], dt.bfloat16, kind="ExternalInput")
    wT_d = nc.dram_tensor("wT", [P, TF], dt.bfloat16, kind="ExternalInput")
    iota_d = nc.dram_tensor("iota", [P, P], dt.bfloat16, kind="ExternalInput")
    xs_d = nc.dram_tensor("xs", [P, nwin * F], dt.float32, kind="ExternalInput")
    srel_d = nc.dram_tensor("srel", [P, TT], dt.bfloat16, kind="ExternalInput")
    gidx_d = nc.dram_tensor("gidx", [P, 8 * TT], dt.int16, kind="ExternalInput")
    y_d = nc.dram_tensor("y", [npc, F], dt.float32, kind="ExternalOutput")
    mtab_d = nc.dram_tensor("mtab", [na * P, F], dt.bfloat16, kind="Internal")

    lo_rows = min(cfg.half, na * P)

    max_tl = max((len(lo) for _, lo, _ in plan.groups), default=0)
    max_th = max((len(hi) for _, _, hi in plan.groups), default=0)
    max_tg = max((len(lo) + len(hi) for _, lo, hi in plan.groups), default=0)

    n_chunks = math.ceil(na / cfg.ca)

    from concourse.tile import add_dep_helper

    with tile.TileContext(nc) as tc:
        import contextlib

        with contextlib.ExitStack() as ctx:
            consts = ctx.enter_context(tc.tile_pool(name="consts", bufs=1))
            a_in = ctx.enter_context(tc.tile_pool(name="a_in", bufs=3))
            a_ps = ctx.enter_context(tc.tile_pool(name="a_ps", bufs=4, space="PSUM"))
            a_sg = ctx.enter_context(tc.tile_pool(name="a_sg", bufs=4))
            a_m = ctx.enter_context(tc.tile_pool(name="a_m", bufs=3))
            b_lo = ctx.enter_context(tc.tile_pool(name="b_lo", bufs=2))
            b_hi = ctx.enter_context(tc.tile_pool(name="b_hi", bufs=2))
            b_oh = ctx.enter_context(tc.tile_pool(name="b_oh", bufs=2))
            b_ps = ctx.enter_context(tc.tile_pool(name="b_ps", bufs=2, space="PSUM"))
            b_out = ctx.enter_context(tc.tile_pool(name="b_out", bufs=2))

            # ---- constants to SBUF ----
            wT_sb = consts.tile([P, TF], dt.bfloat16, tag="wT")
            nc.sync.dma_start(wT_sb[:], wT_d[:, :])
            iota_sb = consts.tile([P, P], dt.bfloat16, tag="iota")
            nc.sync.dma_start(iota_sb[:], iota_d[:, :])
            xs_sb = consts.tile([P, nwin * F], dt.float32, tag="xs")
            nc.sync.dma_start(xs_sb[:], xs_d[:, :])
            srel_sb = consts.tile([P, TT], dt.bfloat16, tag="srel")
            nc.sync.dma_start(srel_sb[:], srel_d[:, :])
            gidx_sb = consts.tile([P, 8 * TT], dt.int16, tag="gidx")
            nc.sync.dma_start(gidx_sb[:], gidx_d[:, :])

            # ---- phase A: M table ----
            mdmas = []
            for ci in range(n_chunks):
                c0 = ci * cfg.ca
                ca = min(cfg.ca, na - c0)
                xt = a_in.tile([P, cfg.ca * P], dt.bfloat16, tag="xt")
                nc.sync.dma_start(xt[:, : ca * P], xT_d[:, c0 * P : (c0 + ca) * P])
                mtile = a_m.tile([P, cfg.ca * F], dt.bfloat16, tag="mtile")
                for k in range(ca):
                    ps = a_ps.tile([P, TF], dt.float32, tag="psA")
                    nc.tensor.matmul(
                        ps[:],
                        lhsT=xt[:, k * P : (k + 1) * P],
                        rhs=wT_sb[:],
                        start=True,
                        stop=True,
                    )
                    sg = a_sg.tile([P, F], dt.float32, tag="sg")
                    nc.scalar.activation(
                        sg[:], ps[:, 0:F], mybir.ActivationFunctionType.Sigmoid
                    )
                    nc.vector.tensor_mul(
                        mtile[:, k * F : (k + 1) * F], ps[:, F:TF], sg[:]
                    )
                out_ap = (
                    mtab_d[c0 * P : (c0 + ca) * P, :]
                    .rearrange("(k p) f -> p k f", p=P)
                )
                mdmas.append(
                    nc.sync.dma_start(
                        out_ap, mtile[:, : ca * F].rearrange("p (k f) -> p k f", f=F)
                    )
                )

            # ---- phase B: gather + one-hot scatter ----
            for ws, lo, hi in plan.groups:
                lob = hib = None
                if lo:
                    t0 = lo[0][1]
                    tl = len(lo)
                    lob = b_lo.tile([P, max(max_tl, 1) * F], dt.bfloat16, tag="lob")
                    g = nc.gpsimd.dma_gather(
                        out_ap=lob[:, : tl * F].rearrange("p (t e) -> p t e", e=F),
                        in_ap=mtab_d[0:lo_rows, :],
                        idxs_ap=gidx_sb[:, 8 * t0 : 8 * (t0 + tl)],
                        num_idxs=tl * P,
                        num_idxs_reg=tl * P,
                        elem_size=F,
                        single_packet=False,
                    )
                    for m in mdmas:
                        add_dep_helper(g.ins, m.ins, reason="mtab RAW")
                if hi:
                    t0 = hi[0][1]
                    th = len(hi)
                    hib = b_hi.tile([P, max(max_th, 1) * F], dt.bfloat16, tag="hib")
                    g = nc.gpsimd.dma_gather(
                        out_ap=hib[:, : th * F].rearrange("p (t e) -> p t e", e=F),
                        in_ap=mtab_d[cfg.half : na * P, :],
                        idxs_ap=gidx_sb[:, 8 * t0 : 8 * (t0 + th)],
                        num_idxs=th * P,
                        num_idxs_reg=th * P,
                        elem_size=F,
                        single_packet=False,
                    )
                    for m in mdmas:
                        add_dep_helper(g.ins, m.ins, reason="mtab RAW")

                # one-hot for the whole group in one DVE op
                tg0 = (lo + hi)[0][1] if (lo or hi) else None
                ntg = len(lo) + len(hi)
                oh = None
                if ntg:
                    oh = b_oh.tile([P, max(max_tg, 1) * P], dt.bfloat16, tag="oh")
                    nc.vector.tensor_tensor(
                        out=oh[:, : ntg * P].rearrange("p (t e) -> p t e", e=P),
                        in0=srel_sb[:, tg0 : tg0 + ntg]
                        .unsqueeze(2)
                        .to_broadcast([P, ntg, P]),
                        in1=iota_sb[:].unsqueeze(1).to_broadcast([P, ntg, P]),
                        op=mybir.AluOpType.is_equal,
                    )

                for w in ws:
                    tiles = plan.win_tiles.get(w, [])
                    rows = min(P, npc - w * P)
                    ot = b_out.tile([P, F], dt.float32, tag="ot")
                    if not tiles:
                        nc.vector.tensor_copy(ot[:], xs_sb[:, w * F : (w + 1) * F])
                    else:
                        ps = b_ps.tile([P, F], dt.float32, tag="psB")
                        for i, (t, h, pos) in enumerate(tiles):
                            buf = hib if h else lob
                            nc.tensor.matmul(
                                ps[:],
                                lhsT=oh[:, (t - tg0) * P : (t - tg0 + 1) * P],
                                rhs=buf[:, pos * F : (pos + 1) * F],
                                start=(i == 0),
                                stop=(i == len(tiles) - 1),
                            )
                        nc.vector.tensor_add(
                            ot[:], ps[:], xs_sb[:, w * F : (w + 1) * F]
                        )
                    nc.sync.dma_start(y_d[w * P : w * P + rows, :], ot[:rows, :])

    nc.compile()
    return nc


def _in_maps(cfg: Cfg, plan: Plan, inp: np.ndarray, W: np.ndarray):
    xT, wT, iota = _host_arrays(cfg, inp, W)
    maps = []
    for c in range(cfg.n_cores):
        maps.append(
            {
                "xT": xT,
                "wT": wT,
                "iota": iota,
                "xs": _xs_core(cfg, inp, c),
                "srel": plan.srel[c],
                "gidx": plan.gidx[c],
            }
        )
    return maps


def _install_ntff_hook():
    """Provide the antenv.axon_hooks shim trn_boot expects, so trace=True
    can capture NTFF profiles. Silently degrades if anything is missing."""
    try:
        import antenv.axon_hooks  # noqa: F401

        return
    except ImportError:
        pass
    try:
        import types

        import antenv

        mod = types.ModuleType("antenv.axon_hooks")
        _hook = [None]
        mod.set_axon_ntff_profile_hook = lambda h: _hook.__setitem__(0, h)
        mod.get_axon_ntff_profile_hook = lambda: _hook[0]
        sys.modules["antenv.axon_hooks"] = mod
        antenv.axon_hooks = mod
        from trn_agent_boot import trn_boot

        mod.set_axon_ntff_profile_hook(
            trn_boot._ntff_profile_via_ctypes("/opt/axon/libaxon_pjrt.so")
        )
    except Exception:
        pass


def kernel(**inputs) -> np.ndarray:
    inp = np.asarray(inputs["input"], np.float32)
    W = np.asarray(inputs["W"], np.float32)
    es = np.asarray(inputs["edge_sources"]).astype(np.int64)
    et = np.asarray(inputs["edge_targets"]).astype(np.int64)

    cfg = Cfg(n_nodes=inp.shape[0])
    plan = _plan(cfg, es, et)
    nc = _build(cfg, plan)

    from concourse.bass_utils import run_bass_kernel_spmd

    if bool(int(os.environ.get("GGC_TRACE", "0"))):
        _install_ntff_hook()
    res = run_bass_kernel_spmd(
        nc,
        _in_maps(cfg, plan, inp, W),
        core_ids=list(range(cfg.n_cores)),
        trace=bool(int(os.environ.get("GGC_TRACE", "0"))),
    )
    out = np.concatenate([res.results[c]["y"] for c in range(cfg.n_cores)], axis=0)
    if bool(int(os.environ.get("GGC_TRACE", "0"))):
        kernel.last_results = res  # stash for test harness
    return out



# revision 3
# speedup vs baseline: 1.4342x; 1.4342x over previous
"""GatedGraphConvolution Trainium2 kernel.

out = input + segment_sum(sigmoid(g) * e, edge_sources)
  where [g|e] = input[edge_targets] @ W.T

Key algebraic fact: the per-edge message depends ONLY on the target node:
  msg_e = M[target_e],  M[n] = sigmoid(x_n @ Wg.T) * (x_n @ We.T)
so we (phase A) compute the dense M table [N, F] once per core, and
(phase B) gather M rows per edge + scatter-add by source via one-hot
matmuls accumulated in PSUM.

Sharding: nodes are sharded by SOURCE across the 8 cores (6250 nodes each);
each core receives exactly the edges whose source is in its slice, so there
are no collectives.  Edges are sorted by 128-node source "window"; each
window's edges accumulate into one PSUM tile via lhsT=onehot matmuls.
The dma_gather int16 index limit (<=32767) is handled by splitting each
window's edges into low-target (< 32768) and high-target tiles and issuing
gathers against two base offsets of the M table.
"""

import math
import os
import sys
from dataclasses import dataclass, field

import numpy as np

if "/opt/trn_rl_repo" not in sys.path:
    sys.path.insert(0, "/opt/trn_rl_repo")

import ml_dtypes

P = 128  # partitions / tile edge
F = 128  # feature dim (OUT_F == IN_F == 128)
TF = 2 * F

BF16 = ml_dtypes.bfloat16


@dataclass
class Cfg:
    n_nodes: int = 50000
    n_cores: int = 8
    half: int = 32768  # int16 index limit boundary (multiple of 128)
    gw: int = 5  # windows per gather group
    ca: int = 16  # node-tiles per phase-A chunk

    @property
    def na(self) -> int:  # node tiles in M table
        return math.ceil(self.n_nodes / P)

    @property
    def npc(self) -> int:  # nodes per core
        assert self.n_nodes % self.n_cores == 0
        return self.n_nodes // self.n_cores

    @property
    def nwin(self) -> int:  # source windows per core
        return math.ceil(self.npc / P)


@dataclass
class Plan:
    """Static schedule shared by all cores + per-core host arrays."""

    T_lo: list  # tiles per (window, low-half), max over cores
    T_hi: list
    groups: list = field(default_factory=list)  # (ws, lo_tiles, hi_tiles)
    tiles_of: dict = field(default_factory=dict)  # (w, half) -> [tile ids]
    win_tiles: dict = field(default_factory=dict)  # w -> [(tile, half, pos_in_buf)]
    TT: int = 0
    # per-core packed arrays
    srel: list = field(default_factory=list)  # [P, TT] bf16
    gidx: list = field(default_factory=list)  # [P, 8*TT] int16


def _make_schedule(cfg: Cfg, T_lo, T_hi) -> Plan:
    plan = Plan(T_lo=T_lo, T_hi=T_hi)
    t = 0
    for g0 in range(0, cfg.nwin, cfg.gw):
        ws = list(range(g0, min(g0 + cfg.gw, cfg.nwin)))
        lo, hi = [], []
        for w in ws:
            for _ in range(T_lo[w]):
                plan.tiles_of.setdefault((w, 0), []).append(t)
                lo.append((w, t))
                t += 1
        for w in ws:
            for _ in range(T_hi[w]):
                plan.tiles_of.setdefault((w, 1), []).append(t)
                hi.append((w, t))
                t += 1
        plan.groups.append((ws, lo, hi))
    plan.TT = t
    # per window: list of (global tile id, half, position within the group's
    # lo/hi gather buffer) in mm2 consumption order
    for ws, lo, hi in plan.groups:
        for w in ws:
            lst = []
            for pos, (w2, t2) in enumerate(lo):
                if w2 == w:
                    lst.append((t2, 0, pos))
            for pos, (w2, t2) in enumerate(hi):
                if w2 == w:
                    lst.append((t2, 1, pos))
            plan.win_tiles[w] = lst
    return plan


def _plan(cfg: Cfg, edge_sources: np.ndarray, edge_targets: np.ndarray) -> Plan:
    src = edge_sources.astype(np.int64)
    tgt = edge_targets.astype(np.int64)
    npc, nwin = cfg.npc, cfg.nwin

    # bucket edges per (core, window, half)
    core = src // npc
    w_all = (src % npc) // P
    srel_all = (src % npc) % P
    hi_all = (tgt >= cfg.half).astype(np.int64)

    counts = np.zeros((cfg.n_cores, nwin, 2), np.int64)
    np.add.at(counts, (core, w_all, hi_all), 1)
    tmax = counts.max(axis=0)  # [nwin, 2]
    T_lo = [int(math.ceil(tmax[w, 0] / P)) for w in range(nwin)]
    T_hi = [int(math.ceil(tmax[w, 1] / P)) for w in range(nwin)]

    plan = _make_schedule(cfg, T_lo, T_hi)

    # pack per-core slot arrays
    order = np.lexsort((hi_all, w_all, core))
    src_s, w_s, srel_s, hi_s, tgt_s = (
        src[order],
        w_all[order],
        srel_all[order],
        hi_all[order],
        tgt[order],
    )
    bounds = {}
    keys = core[order] * (nwin * 2) + w_s * 2 + hi_s
    uniq, starts = np.unique(keys, return_index=True)
    starts = list(starts) + [len(keys)]
    for i, k in enumerate(uniq):
        bounds[int(k)] = (starts[i], starts[i + 1])

    for c in range(cfg.n_cores):
        srel_arr = np.full((plan.TT * P,), 255.0, np.float32)
        gidx_arr = np.zeros((plan.TT * P,), np.int16)
        for w in range(nwin):
            for h in (0, 1):
                k = c * (nwin * 2) + w * 2 + h
                if k not in bounds:
                    continue
                a, b = bounds[k]
                tiles = plan.tiles_of.get((w, h), [])
                assert (b - a) <= len(tiles) * P
                sr = srel_s[a:b]
                tg = tgt_s[a:b] - (cfg.half if h else 0)
                for i in range(b - a):
                    t = tiles[i // P]
                    j = i % P
                    s = t * P + j
                    srel_arr[s] = sr[i]
                    gidx_arr[s] = tg[i]
        srel_host = srel_arr.reshape(plan.TT, P).T.astype(BF16)  # [P, TT]
        g16 = gidx_arr.reshape(plan.TT * 8, 16).T  # [16, TT*8]
        gidx_host = np.tile(g16, (8, 1)).astype(np.int16)  # [P, TT*8]
        plan.srel.append(np.ascontiguousarray(srel_host))
        plan.gidx.append(np.ascontiguousarray(gidx_host))
    return plan


def _host_arrays(cfg: Cfg, inp: np.ndarray, W: np.ndarray):
    """Replicated input arrays: xT (transposed node features), wT, iota."""
    n = cfg.n_nodes
    xT = np.zeros((P, cfg.na * P), np.float32)
    xT[:, :n] = inp.T
    wT = np.ascontiguousarray(W.T)  # [F, 2F]
    iota = np.tile(np.arange(P, dtype=np.float32), (P, 1))
    return (
        np.ascontiguousarray(xT.astype(BF16)),
        np.ascontiguousarray(wT.astype(BF16)),
        np.ascontiguousarray(iota.astype(BF16)),
    )


def _xs_core(cfg: Cfg, inp: np.ndarray, c: int) -> np.ndarray:
    """Per-core input slice packed [P, nwin*F]: xs[p, w*F+f] = x[c*npc+w*P+p, f]."""
    npc, nwin = cfg.npc, cfg.nwin
    sl = np.zeros((nwin * P, F), np.float32)
    sl[:npc] = inp[c * npc : (c + 1) * npc]
    return np.ascontiguousarray(sl.reshape(nwin, P, F).transpose(1, 0, 2).reshape(P, nwin * F))


def _build(cfg: Cfg, plan: Plan, enable_asserts: bool = False):
    import concourse.bacc as bacc
    import concourse.tile as tile
    from concourse import mybir

    nc = bacc.Bacc(
        "TRN2",
        target_bir_lowering=False,
        debug=False,
        enable_asserts=enable_asserts,
        num_devices=cfg.n_cores,
        num_swdge_queues=4,
    )
    dt = mybir.dt

    na, nwin, npc = cfg.na, cfg.nwin, cfg.npc
    TT = plan.TT

    xT_d = nc.dram_tensor("xT", [P, na * P], dt.bfloat16, kind="ExternalInput")
    wT_d = nc.dram_tensor("wT", [P, TF], dt.bfloat16, kind="ExternalInput")
    iota_d = nc.dram_tensor("iota", [P, P], dt.bfloat16, kind="ExternalInput")
    xs_d = nc.dram_tensor("xs", [P, nwin * F], dt.float32, kind="ExternalInput")
    srel_d = nc.dram_tensor("srel", [P, TT], dt.bfloat16, kind="ExternalInput")
    gidx_d = nc.dram_tensor("gidx", [P, 8 * TT], dt.int16, kind="ExternalInput")
    y_d = nc.dram_tensor("y", [npc, F], dt.float32, kind="ExternalOutput")
    mtab_d = nc.dram_tensor("mtab", [na * P, F], dt.bfloat16, kind="Internal")

    lo_rows = min(cfg.half, na * P)

    max_tl = max((len(lo) for _, lo, _ in plan.groups), default=0)
    max_th = max((len(hi) for _, _, hi in plan.groups), default=0)
    max_tg = max((len(lo) + len(hi) for _, lo, hi in plan.groups), default=0)

    n_chunks = math.ceil(na / cfg.ca)

    from concourse.tile import add_dep_helper

    with tile.TileContext(nc) as tc:
        import contextlib

        with contextlib.ExitStack() as ctx:
            consts = ctx.enter_context(tc.tile_pool(name="consts", bufs=1))
            a_in = ctx.enter_context(tc.tile_pool(name="a_in", bufs=3))
            a_ps = ctx.enter_context(tc.tile_pool(name="a_ps", bufs=4, space="PSUM"))
            a_sg = ctx.enter_context(tc.tile_pool(name="a_sg", bufs=4))
            a_m = ctx.enter_context(tc.tile_pool(name="a_m", bufs=3))
            b_lo = ctx.enter_context(tc.tile_pool(name="b_lo", bufs=2))
            b_hi = ctx.enter_context(tc.tile_pool(name="b_hi", bufs=2))
            b_oh = ctx.enter_context(tc.tile_pool(name="b_oh", bufs=2))
            b_ps = ctx.enter_context(tc.tile_pool(name="b_ps", bufs=2, space="PSUM"))
            b_out = ctx.enter_context(tc.tile_pool(name="b_out", bufs=2))

            # ---- constants to SBUF ----
            wT_sb = consts.tile([P, TF], dt.bfloat16, tag="wT")
            nc.sync.dma_start(wT_sb[:], wT_d[:, :])
            iota_sb = consts.tile([P, P], dt.bfloat16, tag="iota")
            nc.sync.dma_start(iota_sb[:], iota_d[:, :])
            xs_sb = consts.tile([P, nwin * F], dt.float32, tag="xs")
            nc.sync.dma_start(xs_sb[:], xs_d[:, :])
            srel_sb = consts.tile([P, TT], dt.bfloat16, tag="srel")
            nc.sync.dma_start(srel_sb[:], srel_d[:, :])
            gidx_sb = consts.tile([P, 8 * TT], dt.int16, tag="gidx")
            nc.sync.dma_start(gidx_sb[:], gidx_d[:, :])

            # ---- phase A: M table ----
            mdmas = []
            for ci in range(n_chunks):
                c0 = ci * cfg.ca
                ca = min(cfg.ca, na - c0)
                xt = a_in.tile([P, cfg.ca * P], dt.bfloat16, tag="xt")
                nc.sync.dma_start(xt[:, : ca * P], xT_d[:, c0 * P : (c0 + ca) * P])
                mtile = a_m.tile([P, cfg.ca * F], dt.bfloat16, tag="mtile")
                for k in range(ca):
                    ps = a_ps.tile([P, TF], dt.float32, tag="psA")
                    nc.tensor.matmul(
                        ps[:],
                        lhsT=xt[:, k * P : (k + 1) * P],
                        rhs=wT_sb[:],
                        start=True,
                        stop=True,
                    )
                    sg = a_sg.tile([P, F], dt.float32, tag="sg")
                    nc.scalar.activation(
                        sg[:], ps[:, 0:F], mybir.ActivationFunctionType.Sigmoid
                    )
                    nc.vector.tensor_mul(
                        mtile[:, k * F : (k + 1) * F], ps[:, F:TF], sg[:]
                    )
                out_ap = (
                    mtab_d[c0 * P : (c0 + ca) * P, :]
                    .rearrange("(k p) f -> p k f", p=P)
                )
                mdmas.append(
                    nc.sync.dma_start(
                        out_ap, mtile[:, : ca * F].rearrange("p (k f) -> p k f", f=F)
                    )
                )

            # ---- phase B: gather + one-hot scatter ----
            gq = 0  # round-robin SWDGE queue so ring drains overlap
            for ws, lo, hi in plan.groups:
                lob = hib = None
                if lo:
                    t0 = lo[0][1]
                    tl = len(lo)
                    lob = b_lo.tile([P, max(max_tl, 1) * F], dt.bfloat16, tag="lob")
                    g = nc.gpsimd.dma_gather(
                        out_ap=lob[:, : tl * F].rearrange("p (t e) -> p t e", e=F),
                        in_ap=mtab_d[0:lo_rows, :],
                        idxs_ap=gidx_sb[:, 8 * t0 : 8 * (t0 + tl)],
                        num_idxs=tl * P,
                        num_idxs_reg=tl * P,
                        elem_size=F,
                        single_packet=False,
                        queue_num=gq % 4,
                    )
                    gq += 1
                    for m in mdmas:
                        add_dep_helper(g.ins, m.ins, reason="mtab RAW")
                if hi:
                    t0 = hi[0][1]
                    th = len(hi)
                    hib = b_hi.tile([P, max(max_th, 1) * F], dt.bfloat16, tag="hib")
                    g = nc.gpsimd.dma_gather(
                        out_ap=hib[:, : th * F].rearrange("p (t e) -> p t e", e=F),
                        in_ap=mtab_d[cfg.half : na * P, :],
                        idxs_ap=gidx_sb[:, 8 * t0 : 8 * (t0 + th)],
                        num_idxs=th * P,
                        num_idxs_reg=th * P,
                        elem_size=F,
                        single_packet=False,
                        queue_num=gq % 4,
                    )
                    gq += 1
                    for m in mdmas:
                        add_dep_helper(g.ins, m.ins, reason="mtab RAW")

                # one-hot for the whole group in one DVE op
                tg0 = (lo + hi)[0][1] if (lo or hi) else None
                ntg = len(lo) + len(hi)
                oh = None
                if ntg:
                    oh = b_oh.tile([P, max(max_tg, 1) * P], dt.bfloat16, tag="oh")
                    nc.vector.tensor_tensor(
                        out=oh[:, : ntg * P].rearrange("p (t e) -> p t e", e=P),
                        in0=srel_sb[:, tg0 : tg0 + ntg]
                        .unsqueeze(2)
                        .to_broadcast([P, ntg, P]),
                        in1=iota_sb[:].unsqueeze(1).to_broadcast([P, ntg, P]),
                        op=mybir.AluOpType.is_equal,
                    )

                for w in ws:
                    tiles = plan.win_tiles.get(w, [])
                    rows = min(P, npc - w * P)
                    ot = b_out.tile([P, F], dt.float32, tag="ot")
                    if not tiles:
                        nc.vector.tensor_copy(ot[:], xs_sb[:, w * F : (w + 1) * F])
                    else:
                        ps = b_ps.tile([P, F], dt.float32, tag="psB")
                        for i, (t, h, pos) in enumerate(tiles):
                            buf = hib if h else lob
                            nc.tensor.matmul(
                                ps[:],
                                lhsT=oh[:, (t - tg0) * P : (t - tg0 + 1) * P],
                                rhs=buf[:, pos * F : (pos + 1) * F],
                                start=(i == 0),
                                stop=(i == len(tiles) - 1),
                            )
                        nc.vector.tensor_add(
                            ot[:], ps[:], xs_sb[:, w * F : (w + 1) * F]
                        )
                    nc.sync.dma_start(y_d[w * P : w * P + rows, :], ot[:rows, :])

    nc.compile()
    return nc


def _in_maps(cfg: Cfg, plan: Plan, inp: np.ndarray, W: np.ndarray):
    xT, wT, iota = _host_arrays(cfg, inp, W)
    maps = []
    for c in range(cfg.n_cores):
        maps.append(
            {
                "xT": xT,
                "wT": wT,
                "iota": iota,
                "xs": _xs_core(cfg, inp, c),
                "srel": plan.srel[c],
                "gidx": plan.gidx[c],
            }
        )
    return maps


def _install_ntff_hook():
    """Provide the antenv.axon_hooks shim trn_boot expects, so trace=True
    can capture NTFF profiles. Silently degrades if anything is missing."""
    try:
        import antenv.axon_hooks  # noqa: F401

        return
    except ImportError:
        pass
    try:
        import types

        import antenv

        mod = types.ModuleType("antenv.axon_hooks")
        _hook = [None]
        mod.set_axon_ntff_profile_hook = lambda h: _hook.__setitem__(0, h)
        mod.get_axon_ntff_profile_hook = lambda: _hook[0]
        sys.modules["antenv.axon_hooks"] = mod
        antenv.axon_hooks = mod
        from trn_agent_boot import trn_boot

        mod.set_axon_ntff_profile_hook(
            trn_boot._ntff_profile_via_ctypes("/opt/axon/libaxon_pjrt.so")
        )
    except Exception:
        pass


def kernel(**inputs) -> np.ndarray:
    inp = np.asarray(inputs["input"], np.float32)
    W = np.asarray(inputs["W"], np.float32)
    es = np.asarray(inputs["edge_sources"]).astype(np.int64)
    et = np.asarray(inputs["edge_targets"]).astype(np.int64)

    cfg = Cfg(n_nodes=inp.shape[0])
    plan = _plan(cfg, es, et)
    nc = _build(cfg, plan)

    from concourse.bass_utils import run_bass_kernel_spmd

    if bool(int(os.environ.get("GGC_TRACE", "0"))):
        _install_ntff_hook()
    res = run_bass_kernel_spmd(
        nc,
        _in_maps(cfg, plan, inp, W),
        core_ids=list(range(cfg.n_cores)),
        trace=bool(int(os.environ.get("GGC_TRACE", "0"))),
    )
    out = np.concatenate([res.results[c]["y"] for c in range(cfg.n_cores)], axis=0)
    if bool(int(os.environ.get("GGC_TRACE", "0"))):
        kernel.last_results = res  # stash for test harness
    return out



# revision 5
# speedup vs baseline: 1.8146x; 1.2653x over previous
"""GatedGraphConvolution Trainium2 kernel.

out = input + segment_sum(sigmoid(g) * e, edge_sources)
  where [g|e] = input[edge_targets] @ W.T

Key algebraic fact: the per-edge message depends ONLY on the target node:
  msg_e = M[target_e],  M[n] = sigmoid(x_n @ Wg.T) * (x_n @ We.T)
so we (phase A) compute the dense M table [N, F] once per core, and
(phase B) gather M rows per edge + scatter-add by source via one-hot
matmuls accumulated in PSUM.

Sharding: nodes are sharded by SOURCE across the 8 cores (6250 nodes each);
each core receives exactly the edges whose source is in its slice, so there
are no collectives.  Edges are sorted by 128-node source "window"; each
window's edges accumulate into one PSUM tile via lhsT=onehot matmuls.
The dma_gather int16 index limit (<=32767) is handled by splitting each
window's edges into low-target (< 32768) and high-target tiles and issuing
gathers against two base offsets of the M table.
"""

import math
import os
import sys
from dataclasses import dataclass, field

import numpy as np

if "/opt/trn_rl_repo" not in sys.path:
    sys.path.insert(0, "/opt/trn_rl_repo")

import ml_dtypes

P = 128  # partitions / tile edge
F = 128  # feature dim (OUT_F == IN_F == 128)
TF = 2 * F

BF16 = ml_dtypes.bfloat16


@dataclass
class Cfg:
    n_nodes: int = 50000
    n_cores: int = 8
    half: int = 32768  # int16 index limit boundary (multiple of 128)
    gw: int = 3  # windows per gather group
    ca: int = 16  # node-tiles per phase-A chunk
    sub: int = 12  # tiles per sub-gather (balances SWDGE queue desc-gen)
    q0w: float = 2.0  # load weight for queue 0 (its desc-gen blocks dispatch)

    @property
    def na(self) -> int:  # node tiles in M table
        return math.ceil(self.n_nodes / P)

    @property
    def npc(self) -> int:  # nodes per core
        assert self.n_nodes % self.n_cores == 0
        return self.n_nodes // self.n_cores

    @property
    def nwin(self) -> int:  # source windows per core
        return math.ceil(self.npc / P)


@dataclass
class Plan:
    """Static schedule shared by all cores + per-core host arrays."""

    T_lo: list  # tiles per (window, low-half), max over cores
    T_hi: list
    groups: list = field(default_factory=list)  # (ws, lo_tiles, hi_tiles)
    tiles_of: dict = field(default_factory=dict)  # (w, half) -> [tile ids]
    win_tiles: dict = field(default_factory=dict)  # w -> [(tile, half, pos_in_buf)]
    TT: int = 0
    # per-core packed arrays
    srel: list = field(default_factory=list)  # [P, TT] bf16
    gidx: list = field(default_factory=list)  # [P, 8*TT] int16


def _make_schedule(cfg: Cfg, T_lo, T_hi) -> Plan:
    plan = Plan(T_lo=T_lo, T_hi=T_hi)
    t = 0
    for g0 in range(0, cfg.nwin, cfg.gw):
        ws = list(range(g0, min(g0 + cfg.gw, cfg.nwin)))
        lo, hi = [], []
        for w in ws:
            for _ in range(T_lo[w]):
                plan.tiles_of.setdefault((w, 0), []).append(t)
                lo.append((w, t))
                t += 1
        for w in ws:
            for _ in range(T_hi[w]):
                plan.tiles_of.setdefault((w, 1), []).append(t)
                hi.append((w, t))
                t += 1
        plan.groups.append((ws, lo, hi))
    plan.TT = t
    # per window: list of (global tile id, half, position within the group's
    # lo/hi gather buffer) in mm2 consumption order
    for ws, lo, hi in plan.groups:
        for w in ws:
            lst = []
            for pos, (w2, t2) in enumerate(lo):
                if w2 == w:
                    lst.append((t2, 0, pos))
            for pos, (w2, t2) in enumerate(hi):
                if w2 == w:
                    lst.append((t2, 1, pos))
            plan.win_tiles[w] = lst
    return plan


def _plan(cfg: Cfg, edge_sources: np.ndarray, edge_targets: np.ndarray) -> Plan:
    src = edge_sources.astype(np.int64)
    tgt = edge_targets.astype(np.int64)
    npc, nwin = cfg.npc, cfg.nwin

    # bucket edges per (core, window, half)
    core = src // npc
    w_all = (src % npc) // P
    srel_all = (src % npc) % P
    hi_all = (tgt >= cfg.half).astype(np.int64)

    counts = np.zeros((cfg.n_cores, nwin, 2), np.int64)
    np.add.at(counts, (core, w_all, hi_all), 1)
    tmax = counts.max(axis=0)  # [nwin, 2]
    T_lo = [int(math.ceil(tmax[w, 0] / P)) for w in range(nwin)]
    T_hi = [int(math.ceil(tmax[w, 1] / P)) for w in range(nwin)]

    plan = _make_schedule(cfg, T_lo, T_hi)

    # pack per-core slot arrays
    order = np.lexsort((hi_all, w_all, core))
    src_s, w_s, srel_s, hi_s, tgt_s = (
        src[order],
        w_all[order],
        srel_all[order],
        hi_all[order],
        tgt[order],
    )
    bounds = {}
    keys = core[order] * (nwin * 2) + w_s * 2 + hi_s
    uniq, starts = np.unique(keys, return_index=True)
    starts = list(starts) + [len(keys)]
    for i, k in enumerate(uniq):
        bounds[int(k)] = (starts[i], starts[i + 1])

    for c in range(cfg.n_cores):
        srel_arr = np.full((plan.TT * P,), 255.0, np.float32)
        gidx_arr = np.zeros((plan.TT * P,), np.int16)
        for w in range(nwin):
            for h in (0, 1):
                k = c * (nwin * 2) + w * 2 + h
                if k not in bounds:
                    continue
                a, b = bounds[k]
                tiles = plan.tiles_of.get((w, h), [])
                assert (b - a) <= len(tiles) * P
                sr = srel_s[a:b]
                tg = tgt_s[a:b] - (cfg.half if h else 0)
                for i in range(b - a):
                    t = tiles[i // P]
                    j = i % P
                    s = t * P + j
                    srel_arr[s] = sr[i]
                    gidx_arr[s] = tg[i]
        srel_host = srel_arr.reshape(plan.TT, P).T.astype(BF16)  # [P, TT]
        g16 = gidx_arr.reshape(plan.TT * 8, 16).T  # [16, TT*8]
        gidx_host = np.tile(g16, (8, 1)).astype(np.int16)  # [P, TT*8]
        plan.srel.append(np.ascontiguousarray(srel_host))
        plan.gidx.append(np.ascontiguousarray(gidx_host))
    return plan


def _host_arrays(cfg: Cfg, inp: np.ndarray, W: np.ndarray):
    """Replicated input arrays: xT (transposed node features), wT, iota."""
    n = cfg.n_nodes
    xT = np.zeros((P, cfg.na * P), np.float32)
    xT[:, :n] = inp.T
    wT = np.ascontiguousarray(W.T)  # [F, 2F]
    iota = np.tile(np.arange(P, dtype=np.float32), (P, 1))
    return (
        np.ascontiguousarray(xT.astype(BF16)),
        np.ascontiguousarray(wT.astype(BF16)),
        np.ascontiguousarray(iota.astype(BF16)),
    )


def _xs_core(cfg: Cfg, inp: np.ndarray, c: int) -> np.ndarray:
    """Per-core input slice packed [P, nwin*F]: xs[p, w*F+f] = x[c*npc+w*P+p, f]."""
    npc, nwin = cfg.npc, cfg.nwin
    sl = np.zeros((nwin * P, F), np.float32)
    sl[:npc] = inp[c * npc : (c + 1) * npc]
    return np.ascontiguousarray(sl.reshape(nwin, P, F).transpose(1, 0, 2).reshape(P, nwin * F))


def _build(cfg: Cfg, plan: Plan, enable_asserts: bool = False):
    import concourse.bacc as bacc
    import concourse.tile as tile
    from concourse import mybir

    nc = bacc.Bacc(
        "TRN2",
        target_bir_lowering=False,
        debug=False,
        enable_asserts=enable_asserts,
        num_devices=cfg.n_cores,
        num_swdge_queues=4,
    )
    dt = mybir.dt

    na, nwin, npc = cfg.na, cfg.nwin, cfg.npc
    TT = plan.TT

    xT_d = nc.dram_tensor("xT", [P, na * P], dt.bfloat16, kind="ExternalInput")
    wT_d = nc.dram_tensor("wT", [P, TF], dt.bfloat16, kind="ExternalInput")
    iota_d = nc.dram_tensor("iota", [P, P], dt.bfloat16, kind="ExternalInput")
    xs_d = nc.dram_tensor("xs", [P, nwin * F], dt.float32, kind="ExternalInput")
    srel_d = nc.dram_tensor("srel", [P, TT], dt.bfloat16, kind="ExternalInput")
    gidx_d = nc.dram_tensor("gidx", [P, 8 * TT], dt.int16, kind="ExternalInput")
    y_d = nc.dram_tensor("y", [npc, F], dt.float32, kind="ExternalOutput")
    mtab_d = nc.dram_tensor("mtab", [na * P, F], dt.bfloat16, kind="Internal")

    lo_rows = min(cfg.half, na * P)

    max_tl = max((len(lo) for _, lo, _ in plan.groups), default=0)
    max_th = max((len(hi) for _, _, hi in plan.groups), default=0)
    max_tg = max((len(lo) + len(hi) for _, lo, hi in plan.groups), default=0)

    n_chunks = math.ceil(na / cfg.ca)

    from concourse.tile import add_dep_helper

    S = cfg.sub

    with tile.TileContext(nc) as tc:
        import contextlib

        with contextlib.ExitStack() as ctx:
            consts = ctx.enter_context(tc.tile_pool(name="consts", bufs=1))
            a_in = ctx.enter_context(tc.tile_pool(name="a_in", bufs=3))
            a_ps = ctx.enter_context(tc.tile_pool(name="a_ps", bufs=4, space="PSUM"))
            a_sg = ctx.enter_context(tc.tile_pool(name="a_sg", bufs=4))
            a_m = ctx.enter_context(tc.tile_pool(name="a_m", bufs=3))
            b_lo = ctx.enter_context(tc.tile_pool(name="b_lo", bufs=8))
            b_hi = ctx.enter_context(tc.tile_pool(name="b_hi", bufs=6))
            b_oh = ctx.enter_context(tc.tile_pool(name="b_oh", bufs=2))
            b_ps = ctx.enter_context(tc.tile_pool(name="b_ps", bufs=3, space="PSUM"))
            b_out = ctx.enter_context(tc.tile_pool(name="b_out", bufs=4))

            # ---- constants to SBUF ----
            wT_sb = consts.tile([P, TF], dt.bfloat16, tag="wT")
            nc.sync.dma_start(wT_sb[:], wT_d[:, :])
            iota_sb = consts.tile([P, P], dt.bfloat16, tag="iota")
            nc.sync.dma_start(iota_sb[:], iota_d[:, :])
            xs_sb = consts.tile([P, nwin * F], dt.float32, tag="xs")
            nc.sync.dma_start(xs_sb[:], xs_d[:, :])
            srel_sb = consts.tile([P, TT], dt.bfloat16, tag="srel")
            nc.sync.dma_start(srel_sb[:], srel_d[:, :])
            gidx_sb = consts.tile([P, 8 * TT], dt.int16, tag="gidx")
            nc.sync.dma_start(gidx_sb[:], gidx_d[:, :])

            # ---- phase A: M table (2 node-tiles batched per PSUM bank) ----
            mdmas = []
            last_a_op = None
            for ci in range(n_chunks):
                c0 = ci * cfg.ca
                ca = min(cfg.ca, na - c0)
                xt = a_in.tile([P, cfg.ca * P], dt.bfloat16, tag="xt")
                nc.sync.dma_start(xt[:, : ca * P], xT_d[:, c0 * P : (c0 + ca) * P])
                mtile = a_m.tile([P, cfg.ca * F], dt.bfloat16, tag="mtile")
                for k0 in range(0, ca, 2):
                    kk = min(2, ca - k0)
                    ps = a_ps.tile([P, 2 * TF], dt.float32, tag="psA")
                    for j in range(kk):
                        nc.tensor.matmul(
                            ps[:, j * TF : (j + 1) * TF],
                            lhsT=xt[:, (k0 + j) * P : (k0 + j + 1) * P],
                            rhs=wT_sb[:],
                            start=True,
                            stop=True,
                        )
                    psv = ps[:, : kk * TF].rearrange("p (k tf) -> p k tf", tf=TF)
                    sg = a_sg.tile([P, 2 * F], dt.float32, tag="sg")
                    sgv = sg[:, : kk * F].rearrange("p (k f) -> p k f", f=F)
                    nc.scalar.activation(
                        sgv, psv[:, :, 0:F], mybir.ActivationFunctionType.Sigmoid
                    )
                    mv = mtile[:, k0 * F : (k0 + kk) * F].rearrange(
                        "p (k f) -> p k f", f=F
                    )
                    last_a_op = nc.vector.tensor_tensor(
                        out=mv, in0=psv[:, :, F:TF], in1=sgv,
                        op=mybir.AluOpType.mult,
                    )
                out_ap = (
                    mtab_d[c0 * P : (c0 + ca) * P, :]
                    .rearrange("(k p) f -> p k f", p=P)
                )
                mdmas.append(
                    nc.sync.dma_start(
                        out_ap, mtile[:, : ca * F].rearrange("p (k f) -> p k f", f=F)
                    )
                )

            # ---- phase B: sub-gathers balanced over SWDGE queues ----
            # Desc-gen runs on a per-queue Q7 core pair (~7.7ns/row); queue 0's
            # gen blocks engine dispatch, so it gets less load and goes last.
            qload = [0.0, 0.0, 0.0, 0.0]
            first_iseq = [True]
            for ws, lo, hi in plan.groups:
                # split the group's lo/hi tile runs into sub-gathers
                subs = []  # (half, pool, base_ap, t_start, pos0, ln)
                if lo:
                    t0, n = lo[0][1], len(lo)
                    for j0 in range(0, n, S):
                        subs.append((0, t0 + j0, j0, min(S, n - j0)))
                if hi:
                    t0, n = hi[0][1], len(hi)
                    for j0 in range(0, n, S):
                        subs.append((1, t0 + j0, j0, min(S, n - j0)))
                assigned = []
                for h, tstart, pos0, ln in subs:
                    q = min(
                        range(4),
                        key=lambda qq: (qload[qq] + ln)
                        * (cfg.q0w if qq == 0 else 1.0),
                    )
                    qload[q] += ln
                    assigned.append((q, h, tstart, pos0, ln))
                assigned.sort(key=lambda a: a[0] == 0)  # queue 0 last
                group_subs = {0: {}, 1: {}}  # half -> {pos0: tile handle}
                for q, h, tstart, pos0, ln in assigned:
                    pool = b_hi if h else b_lo
                    buf = pool.tile([P, S * F], dt.bfloat16, tag="hib" if h else "lob")
                    group_subs[h][pos0] = buf
                    in_ap = (
                        mtab_d[cfg.half : na * P, :] if h else mtab_d[0:lo_rows, :]
                    )
                    g = nc.gpsimd.dma_gather(
                        out_ap=buf[:, : ln * F].rearrange("p (t e) -> p t e", e=F),
                        in_ap=in_ap,
                        idxs_ap=gidx_sb[:, 8 * tstart : 8 * (tstart + ln)],
                        num_idxs=ln * P,
                        num_idxs_reg=ln * P,
                        elem_size=F,
                        single_packet=False,
                        queue_num=q,
                    )
                    for m in mdmas:
                        add_dep_helper(g.ins, m.ins, reason="mtab RAW")

                # one-hot for the whole group in one DVE op
                tg0 = (lo + hi)[0][1] if (lo or hi) else None
                ntg = len(lo) + len(hi)
                oh = None
                if ntg:
                    oh = b_oh.tile([P, max(max_tg, 1) * P], dt.bfloat16, tag="oh")
                    iseq = nc.vector.tensor_tensor(
                        out=oh[:, : ntg * P].rearrange("p (t e) -> p t e", e=P),
                        in0=srel_sb[:, tg0 : tg0 + ntg]
                        .unsqueeze(2)
                        .to_broadcast([P, ntg, P]),
                        in1=iota_sb[:].unsqueeze(1).to_broadcast([P, ntg, P]),
                        op=mybir.AluOpType.is_equal,
                    )
                    if first_iseq[0] and last_a_op is not None:
                        # keep DVE queue ordered: phase-A muls before one-hots
                        add_dep_helper(iseq.ins, last_a_op.ins, reason="DVE order")
                        first_iseq[0] = False

                for w in ws:
                    tiles = plan.win_tiles.get(w, [])
                    rows = min(P, npc - w * P)
                    ot = b_out.tile([P, F], dt.float32, tag="ot")
                    if not tiles:
                        nc.vector.tensor_copy(ot[:], xs_sb[:, w * F : (w + 1) * F])
                    else:
                        ps = b_ps.tile([P, F], dt.float32, tag="psB")
                        for i, (t, h, pos) in enumerate(tiles):
                            buf = group_subs[h][(pos // S) * S]
                            col = pos % S
                            nc.tensor.matmul(
                                ps[:],
                                lhsT=oh[:, (t - tg0) * P : (t - tg0 + 1) * P],
                                rhs=buf[:, col * F : (col + 1) * F],
                                start=(i == 0),
                                stop=(i == len(tiles) - 1),
                            )
                        nc.vector.tensor_add(
                            ot[:], ps[:], xs_sb[:, w * F : (w + 1) * F]
                        )
                    nc.sync.dma_start(y_d[w * P : w * P + rows, :], ot[:rows, :])

    nc.compile()
    return nc


def _in_maps(cfg: Cfg, plan: Plan, inp: np.ndarray, W: np.ndarray):
    xT, wT, iota = _host_arrays(cfg, inp, W)
    maps = []
    for c in range(cfg.n_cores):
        maps.append(
            {
                "xT": xT,
                "wT": wT,
                "iota": iota,
                "xs": _xs_core(cfg, inp, c),
                "srel": plan.srel[c],
                "gidx": plan.gidx[c],
            }
        )
    return maps


def _install_ntff_hook():
    """Provide the antenv.axon_hooks shim trn_boot expects, so trace=True
    can capture NTFF profiles. Silently degrades if anything is missing."""
    try:
        import antenv.axon_hooks  # noqa: F401

        return
    except ImportError:
        pass
    try:
        import types

        import antenv

        mod = types.ModuleType("antenv.axon_hooks")
        _hook = [None]
        mod.set_axon_ntff_profile_hook = lambda h: _hook.__setitem__(0, h)
        mod.get_axon_ntff_profile_hook = lambda: _hook[0]
        sys.modules["antenv.axon_hooks"] = mod
        antenv.axon_hooks = mod
        from trn_agent_boot import trn_boot

        mod.set_axon_ntff_profile_hook(
            trn_boot._ntff_profile_via_ctypes("/opt/axon/libaxon_pjrt.so")
        )
    except Exception:
        pass


def kernel(**inputs) -> np.ndarray:
    inp = np.asarray(inputs["input"], np.float32)
    W = np.asarray(inputs["W"], np.float32)
    es = np.asarray(inputs["edge_sources"]).astype(np.int64)
    et = np.asarray(inputs["edge_targets"]).astype(np.int64)

    cfg = Cfg(n_nodes=inp.shape[0])
    plan = _plan(cfg, es, et)
    nc = _build(cfg, plan)

    from concourse.bass_utils import run_bass_kernel_spmd

    if bool(int(os.environ.get("GGC_TRACE", "0"))):
        _install_ntff_hook()
    res = run_bass_kernel_spmd(
        nc,
        _in_maps(cfg, plan, inp, W),
        core_ids=list(range(cfg.n_cores)),
        trace=bool(int(os.environ.get("GGC_TRACE", "0"))),
    )
    out = np.concatenate([res.results[c]["y"] for c in range(cfg.n_cores)], axis=0)
    if bool(int(os.environ.get("GGC_TRACE", "0"))):
        kernel.last_results = res  # stash for test harness
    return out



# revision 10
# speedup vs baseline: 2.0036x; 1.1042x over previous
"""GatedGraphConvolution Trainium2 kernel.

out = input + segment_sum(sigmoid(g) * e, edge_sources)
  where [g|e] = input[edge_targets] @ W.T

Key algebraic fact: the per-edge message depends ONLY on the target node:
  msg_e = M[target_e],  M[n] = sigmoid(x_n @ Wg.T) * (x_n @ We.T)
so we (phase A) compute the dense M table [N, F] once per core, and
(phase B) gather M rows per edge + scatter-add by source via one-hot
matmuls accumulated in PSUM.

Sharding: nodes are sharded by SOURCE across the 8 cores (6250 nodes each);
each core receives exactly the edges whose source is in its slice, so there
are no collectives.  Edges are sorted by 128-node source "window"; each
window's edges accumulate into one PSUM tile via lhsT=onehot matmuls.
The dma_gather int16 index limit (<=32767) is handled by splitting each
window's edges into low-target (< 32768) and high-target tiles and issuing
gathers against two base offsets of the M table.
"""

import math
import os
import sys
from dataclasses import dataclass, field

import numpy as np

if "/opt/trn_rl_repo" not in sys.path:
    sys.path.insert(0, "/opt/trn_rl_repo")

import ml_dtypes

P = 128  # partitions / tile edge
F = 128  # feature dim (OUT_F == IN_F == 128)
TF = 2 * F

BF16 = ml_dtypes.bfloat16


@dataclass
class Cfg:
    n_nodes: int = 50000
    n_cores: int = 8
    # mtab row split for the two gather passes; both segments must stay under
    # the 32768 int16 gather-index limit, and the boundary must align to a
    # phase-A chunk (ca*P rows) so lo gathers only depend on seg0 writes.
    half: int = 26624
    gw: int = 3  # windows per gather group
    ca: int = 16  # node-tiles per phase-A chunk
    sub: int = 12  # tiles per sub-gather (balances SWDGE queue desc-gen)

    @property
    def na(self) -> int:  # node tiles in M table
        return math.ceil(self.n_nodes / P)

    @property
    def npc(self) -> int:  # nodes per core
        assert self.n_nodes % self.n_cores == 0
        return self.n_nodes // self.n_cores

    @property
    def nwin(self) -> int:  # source windows per core
        return math.ceil(self.npc / P)


@dataclass
class Plan:
    """Static schedule shared by all cores + per-core host arrays."""

    T_lo: list  # tiles per (window, low-half), max over cores
    T_hi: list
    groups: list = field(default_factory=list)  # (ws, lo_tiles, hi_tiles)
    tiles_of: dict = field(default_factory=dict)  # (w, half) -> [tile ids]
    win_tiles: dict = field(default_factory=dict)  # w -> [(tile, half, pos_in_buf)]
    TT: int = 0
    # per-core packed arrays
    srel: list = field(default_factory=list)  # [P, TT] bf16
    gidx: list = field(default_factory=list)  # [P, 8*TT] int16


def _make_schedule(cfg: Cfg, T_lo, T_hi) -> Plan:
    plan = Plan(T_lo=T_lo, T_hi=T_hi)
    t = 0
    for g0 in range(0, cfg.nwin, cfg.gw):
        ws = list(range(g0, min(g0 + cfg.gw, cfg.nwin)))
        lo, hi = [], []
        for w in ws:
            for _ in range(T_lo[w]):
                plan.tiles_of.setdefault((w, 0), []).append(t)
                lo.append((w, t))
                t += 1
        for w in ws:
            for _ in range(T_hi[w]):
                plan.tiles_of.setdefault((w, 1), []).append(t)
                hi.append((w, t))
                t += 1
        plan.groups.append((ws, lo, hi))
    plan.TT = t
    # per window and half: list of (global tile id, position within the
    # group's lo/hi gather buffer) in consumption order
    for ws, lo, hi in plan.groups:
        for w in ws:
            plan.win_tiles[(w, 0)] = [
                (t2, pos) for pos, (w2, t2) in enumerate(lo) if w2 == w
            ]
            plan.win_tiles[(w, 1)] = [
                (t2, pos) for pos, (w2, t2) in enumerate(hi) if w2 == w
            ]
    return plan


def _plan(cfg: Cfg, edge_sources: np.ndarray, edge_targets: np.ndarray) -> Plan:
    src = edge_sources.astype(np.int64)
    tgt = edge_targets.astype(np.int64)
    npc, nwin = cfg.npc, cfg.nwin

    # bucket edges per (core, window, half)
    core = src // npc
    w_all = (src % npc) // P
    srel_all = (src % npc) % P
    hi_all = (tgt >= cfg.half).astype(np.int64)

    counts = np.zeros((cfg.n_cores, nwin, 2), np.int64)
    np.add.at(counts, (core, w_all, hi_all), 1)
    tmax = counts.max(axis=0)  # [nwin, 2]
    T_lo = [int(math.ceil(tmax[w, 0] / P)) for w in range(nwin)]
    T_hi = [int(math.ceil(tmax[w, 1] / P)) for w in range(nwin)]

    plan = _make_schedule(cfg, T_lo, T_hi)

    # pack per-core slot arrays
    order = np.lexsort((hi_all, w_all, core))
    src_s, w_s, srel_s, hi_s, tgt_s = (
        src[order],
        w_all[order],
        srel_all[order],
        hi_all[order],
        tgt[order],
    )
    bounds = {}
    keys = core[order] * (nwin * 2) + w_s * 2 + hi_s
    uniq, starts = np.unique(keys, return_index=True)
    starts = list(starts) + [len(keys)]
    for i, k in enumerate(uniq):
        bounds[int(k)] = (starts[i], starts[i + 1])

    for c in range(cfg.n_cores):
        srel_arr = np.full((plan.TT * P,), 255.0, np.float32)
        gidx_arr = np.zeros((plan.TT * P,), np.int16)
        for w in range(nwin):
            for h in (0, 1):
                k = c * (nwin * 2) + w * 2 + h
                if k not in bounds:
                    continue
                a, b = bounds[k]
                tiles = plan.tiles_of.get((w, h), [])
                assert (b - a) <= len(tiles) * P
                sr = srel_s[a:b]
                tg = tgt_s[a:b] - (cfg.half if h else 0)
                for i in range(b - a):
                    t = tiles[i // P]
                    j = i % P
                    s = t * P + j
                    srel_arr[s] = sr[i]
                    gidx_arr[s] = tg[i]
        srel_host = srel_arr.reshape(plan.TT, P).T.astype(BF16)  # [P, TT]
        g16 = gidx_arr.reshape(plan.TT * 8, 16).T  # [16, TT*8]
        gidx_host = np.tile(g16, (8, 1)).astype(np.int16)  # [P, TT*8]
        plan.srel.append(np.ascontiguousarray(srel_host))
        plan.gidx.append(np.ascontiguousarray(gidx_host))
    return plan


def _host_arrays(cfg: Cfg, inp: np.ndarray, W: np.ndarray):
    """Replicated input arrays: xT (transposed node features), wT, iota."""
    n = cfg.n_nodes
    xT = np.zeros((P, cfg.na * P), np.float32)
    xT[:, :n] = inp.T
    wT = np.ascontiguousarray(W.T)  # [F, 2F]
    iota = np.tile(np.arange(P, dtype=np.float32), (P, 1))
    return (
        np.ascontiguousarray(xT.astype(BF16)),
        np.ascontiguousarray(wT.astype(BF16)),
        np.ascontiguousarray(iota.astype(BF16)),
    )


def _xs_core(cfg: Cfg, inp: np.ndarray, c: int) -> np.ndarray:
    """Per-core input slice packed [P, nwin*F]: xs[p, w*F+f] = x[c*npc+w*P+p, f]."""
    npc, nwin = cfg.npc, cfg.nwin
    sl = np.zeros((nwin * P, F), np.float32)
    sl[:npc] = inp[c * npc : (c + 1) * npc]
    return np.ascontiguousarray(sl.reshape(nwin, P, F).transpose(1, 0, 2).reshape(P, nwin * F))


def _build(cfg: Cfg, plan: Plan, enable_asserts: bool = False):
    import concourse.bacc as bacc
    import concourse.tile as tile
    from concourse import mybir

    nc = bacc.Bacc(
        "TRN2",
        target_bir_lowering=False,
        debug=False,
        enable_asserts=enable_asserts,
        num_devices=cfg.n_cores,
        num_swdge_queues=4,
    )
    dt = mybir.dt

    na, nwin, npc = cfg.na, cfg.nwin, cfg.npc
    TT = plan.TT

    xT_d = nc.dram_tensor("xT", [P, na * P], dt.bfloat16, kind="ExternalInput")
    wT_d = nc.dram_tensor("wT", [P, TF], dt.bfloat16, kind="ExternalInput")
    iota_d = nc.dram_tensor("iota", [P, P], dt.bfloat16, kind="ExternalInput")
    xs_d = nc.dram_tensor("xs", [P, nwin * F], dt.float32, kind="ExternalInput")
    srel_d = nc.dram_tensor("srel", [P, TT], dt.bfloat16, kind="ExternalInput")
    gidx_d = nc.dram_tensor("gidx", [P, 8 * TT], dt.int16, kind="ExternalInput")
    y_d = nc.dram_tensor("y", [npc, F], dt.float32, kind="ExternalOutput")
    mtab_d = nc.dram_tensor("mtab", [na * P, F], dt.bfloat16, kind="Internal")

    lo_rows = min(cfg.half, na * P)

    max_tl = max((len(lo) for _, lo, _ in plan.groups), default=0)
    max_th = max((len(hi) for _, _, hi in plan.groups), default=0)
    max_tg = max((len(lo) + len(hi) for _, lo, hi in plan.groups), default=0)

    n_chunks = math.ceil(na / cfg.ca)

    from concourse.tile import add_dep_helper

    S = cfg.sub

    with tile.TileContext(nc) as tc:
        import contextlib

        with contextlib.ExitStack() as ctx:
            consts = ctx.enter_context(tc.tile_pool(name="consts", bufs=1))
            a_in = ctx.enter_context(tc.tile_pool(name="a_in", bufs=3))
            a_ps = ctx.enter_context(tc.tile_pool(name="a_ps", bufs=4, space="PSUM"))
            a_sg = ctx.enter_context(tc.tile_pool(name="a_sg", bufs=4))
            a_m = ctx.enter_context(tc.tile_pool(name="a_m", bufs=3))
            b_lo = ctx.enter_context(tc.tile_pool(name="b_lo", bufs=8))
            b_hi = ctx.enter_context(tc.tile_pool(name="b_hi", bufs=6))
            b_oh = ctx.enter_context(tc.tile_pool(name="b_oh", bufs=2))
            b_ps = ctx.enter_context(tc.tile_pool(name="b_ps", bufs=3, space="PSUM"))
            b_out = ctx.enter_context(tc.tile_pool(name="b_out", bufs=4))

            # ---- constants to SBUF ----
            # tiny consts on the SP queue (shared with mtab writes); bulky
            # consts + xt reads go on the Activation HWDGE queue so the first
            # matmul's inputs arrive immediately.
            wT_sb = consts.tile([P, TF], dt.bfloat16, tag="wT")
            nc.sync.dma_start(wT_sb[:], wT_d[:, :])
            iota_sb = consts.tile([P, P], dt.bfloat16, tag="iota")
            nc.sync.dma_start(iota_sb[:], iota_d[:, :])
            xs_sb = consts.tile([P, nwin * F], dt.float32, tag="xs")
            srel_sb = consts.tile([P, TT], dt.bfloat16, tag="srel")
            gidx_sb = consts.tile([P, 8 * TT], dt.int16, tag="gidx")

            # ---- phase A: M table (2 node-tiles batched per PSUM bank) ----
            mdmas = []
            last_a_op = None
            for ci in range(n_chunks):
                c0 = ci * cfg.ca
                ca = min(cfg.ca, na - c0)
                xt = a_in.tile([P, cfg.ca * P], dt.bfloat16, tag="xt")
                nc.scalar.dma_start(xt[:, : ca * P], xT_d[:, c0 * P : (c0 + ca) * P])
                mtile = a_m.tile([P, cfg.ca * F], dt.bfloat16, tag="mtile")
                for k0 in range(0, ca, 2):
                    kk = min(2, ca - k0)
                    ps = a_ps.tile([P, 2 * TF], dt.float32, tag="psA")
                    for j in range(kk):
                        nc.tensor.matmul(
                            ps[:, j * TF : (j + 1) * TF],
                            lhsT=xt[:, (k0 + j) * P : (k0 + j + 1) * P],
                            rhs=wT_sb[:],
                            start=True,
                            stop=True,
                        )
                    psv = ps[:, : kk * TF].rearrange("p (k tf) -> p k tf", tf=TF)
                    sg = a_sg.tile([P, 2 * F], dt.float32, tag="sg")
                    sgv = sg[:, : kk * F].rearrange("p (k f) -> p k f", f=F)
                    nc.scalar.activation(
                        sgv, psv[:, :, 0:F], mybir.ActivationFunctionType.Sigmoid
                    )
                    mv = mtile[:, k0 * F : (k0 + kk) * F].rearrange(
                        "p (k f) -> p k f", f=F
                    )
                    last_a_op = nc.vector.tensor_tensor(
                        out=mv, in0=psv[:, :, F:TF], in1=sgv,
                        op=mybir.AluOpType.mult,
                    )
                out_ap = (
                    mtab_d[c0 * P : (c0 + ca) * P, :]
                    .rearrange("(k p) f -> p k f", p=P)
                )
                mdmas.append(
                    nc.sync.dma_start(
                        out_ap, mtile[:, : ca * F].rearrange("p (k f) -> p k f", f=F)
                    )
                )

            # bulky consts follow the xt chunks on the Activation queue; they
            # are only needed once phase B starts.
            nc.scalar.dma_start(gidx_sb[:], gidx_d[:, :])
            nc.scalar.dma_start(srel_sb[:], srel_d[:, :])
            nc.scalar.dma_start(xs_sb[:], xs_d[:, :])

            # ---- phase B: two passes (lo rows of mtab, then hi rows) ----
            # Pass 1 gathers depend only on seg0 mtab chunks, so their Q7
            # desc-gen + drains overlap the tail of phase A. Partial window
            # sums (lo contribution + input) park in SBUF until pass 2.
            seg_chunks = cfg.half // (cfg.ca * P)
            assert seg_chunks * cfg.ca * P == cfg.half
            mdma_seg = {0: mdmas[:seg_chunks], 1: mdmas[seg_chunks:]}
            part = consts.tile([P, nwin * F], dt.float32, tag="part")

            qi = [0]  # strict round-robin SWDGE queue counter
            first_iseq = [True]

            def emit_gathers(h, seq):
                """Emit sub-gathers for one group's lo or hi tile run."""
                subs = {}
                if not seq:
                    return subs
                t0, n = seq[0][1], len(seq)
                for j0 in range(0, n, S):
                    ln = min(S, n - j0)
                    pool = b_hi if h else b_lo
                    buf = pool.tile(
                        [P, S * F], dt.bfloat16, tag="hib" if h else "lob"
                    )
                    subs[j0] = buf
                    in_ap = (
                        mtab_d[cfg.half : na * P, :] if h else mtab_d[0:lo_rows, :]
                    )
                    g = nc.gpsimd.dma_gather(
                        out_ap=buf[:, : ln * F].rearrange("p (t e) -> p t e", e=F),
                        in_ap=in_ap,
                        idxs_ap=gidx_sb[:, 8 * (t0 + j0) : 8 * (t0 + j0 + ln)],
                        num_idxs=ln * P,
                        num_idxs_reg=ln * P,
                        elem_size=F,
                        single_packet=False,
                        queue_num=qi[0] % 4,
                    )
                    qi[0] += 1
                    for m in mdma_seg[h]:
                        add_dep_helper(g.ins, m.ins, reason="mtab RAW")
                return subs

            def emit_onehot(seq):
                if not seq:
                    return None, 0
                tg0, ntg = seq[0][1], len(seq)
                oh = b_oh.tile([P, max(max_tg, 1) * P], dt.bfloat16, tag="oh")
                iseq = nc.vector.tensor_tensor(
                    out=oh[:, : ntg * P].rearrange("p (t e) -> p t e", e=P),
                    in0=srel_sb[:, tg0 : tg0 + ntg]
                    .unsqueeze(2)
                    .to_broadcast([P, ntg, P]),
                    in1=iota_sb[:].unsqueeze(1).to_broadcast([P, ntg, P]),
                    op=mybir.AluOpType.is_equal,
                )
                if first_iseq[0] and last_a_op is not None:
                    # keep DVE queue ordered: phase-A muls before one-hots
                    add_dep_helper(iseq.ins, last_a_op.ins, reason="DVE order")
                    first_iseq[0] = False
                return oh, tg0

            def accumulate(w, h, oh, tg0, subs, out_sl, add_sl):
                """ps = sum of window tiles; out_sl = ps + add_sl (or copy)."""
                tiles = plan.win_tiles.get((w, h), [])
                if not tiles:
                    nc.vector.tensor_copy(out_sl, add_sl)
                    return
                ps = b_ps.tile([P, F], dt.float32, tag="psB")
                for i, (t, pos) in enumerate(tiles):
                    buf = subs[(pos // S) * S]
                    col = pos % S
                    nc.tensor.matmul(
                        ps[:],
                        lhsT=oh[:, (t - tg0) * P : (t - tg0 + 1) * P],
                        rhs=buf[:, col * F : (col + 1) * F],
                        start=(i == 0),
                        stop=(i == len(tiles) - 1),
                    )
                nc.vector.tensor_add(out_sl, ps[:], add_sl)

            # pass 1: lo rows -> part[w] = sum_lo + xs[w]
            for ws, lo, hi in plan.groups:
                subs = emit_gathers(0, lo)
                oh, tg0 = emit_onehot(lo)
                for w in ws:
                    accumulate(
                        w, 0, oh, tg0, subs,
                        part[:, w * F : (w + 1) * F],
                        xs_sb[:, w * F : (w + 1) * F],
                    )

            # pass 2: hi rows -> y[w] = sum_hi + part[w]
            for ws, lo, hi in plan.groups:
                subs = emit_gathers(1, hi)
                oh, tg0 = emit_onehot(hi)
                for w in ws:
                    rows = min(P, npc - w * P)
                    ot = b_out.tile([P, F], dt.float32, tag="ot")
                    accumulate(
                        w, 1, oh, tg0, subs, ot[:], part[:, w * F : (w + 1) * F]
                    )
                    nc.sync.dma_start(y_d[w * P : w * P + rows, :], ot[:rows, :])

    nc.compile()
    return nc


def _in_maps(cfg: Cfg, plan: Plan, inp: np.ndarray, W: np.ndarray):
    xT, wT, iota = _host_arrays(cfg, inp, W)
    maps = []
    for c in range(cfg.n_cores):
        maps.append(
            {
                "xT": xT,
                "wT": wT,
                "iota": iota,
                "xs": _xs_core(cfg, inp, c),
                "srel": plan.srel[c],
                "gidx": plan.gidx[c],
            }
        )
    return maps


def _install_ntff_hook():
    """Provide the antenv.axon_hooks shim trn_boot expects, so trace=True
    can capture NTFF profiles. Silently degrades if anything is missing."""
    try:
        import antenv.axon_hooks  # noqa: F401

        return
    except ImportError:
        pass
    try:
        import types

        import antenv

        mod = types.ModuleType("antenv.axon_hooks")
        _hook = [None]
        mod.set_axon_ntff_profile_hook = lambda h: _hook.__setitem__(0, h)
        mod.get_axon_ntff_profile_hook = lambda: _hook[0]
        sys.modules["antenv.axon_hooks"] = mod
        antenv.axon_hooks = mod
        from trn_agent_boot import trn_boot

        mod.set_axon_ntff_profile_hook(
            trn_boot._ntff_profile_via_ctypes("/opt/axon/libaxon_pjrt.so")
        )
    except Exception:
        pass


def kernel(**inputs) -> np.ndarray:
    inp = np.asarray(inputs["input"], np.float32)
    W = np.asarray(inputs["W"], np.float32)
    es = np.asarray(inputs["edge_sources"]).astype(np.int64)
    et = np.asarray(inputs["edge_targets"]).astype(np.int64)

    cfg = Cfg(n_nodes=inp.shape[0])
    plan = _plan(cfg, es, et)
    nc = _build(cfg, plan)

    from concourse.bass_utils import run_bass_kernel_spmd

    if bool(int(os.environ.get("GGC_TRACE", "0"))):
        _install_ntff_hook()
    res = run_bass_kernel_spmd(
        nc,
        _in_maps(cfg, plan, inp, W),
        core_ids=list(range(cfg.n_cores)),
        trace=bool(int(os.environ.get("GGC_TRACE", "0"))),
    )
    out = np.concatenate([res.results[c]["y"] for c in range(cfg.n_cores)], axis=0)
    if bool(int(os.environ.get("GGC_TRACE", "0"))):
        kernel.last_results = res  # stash for test harness
    return out



# revision 14
# speedup vs baseline: 2.2591x; 1.1275x over previous
"""GatedGraphConvolution Trainium2 kernel.

out = input + segment_sum(sigmoid(g) * e, edge_sources)
  where [g|e] = input[edge_targets] @ W.T

Key algebraic fact: the per-edge message depends ONLY on the target node:
  msg_e = M[target_e],  M[n] = sigmoid(x_n @ Wg.T) * (x_n @ We.T)
so we (phase A) compute the dense M table [N, F] once per core, and
(phase B) gather M rows per edge + scatter-add by source via one-hot
matmuls accumulated in PSUM.

Sharding: nodes are sharded by SOURCE across the 8 cores (6250 nodes each);
each core receives exactly the edges whose source is in its slice, so there
are no collectives.  Edges are sorted by 128-node source "window"; each
window's edges accumulate into one PSUM tile via lhsT=onehot matmuls.
The dma_gather int16 index limit (<=32767) is handled by splitting each
window's edges into low-target (< 32768) and high-target tiles and issuing
gathers against two base offsets of the M table.
"""

import math
import os
import sys
from dataclasses import dataclass, field

import numpy as np

if "/opt/trn_rl_repo" not in sys.path:
    sys.path.insert(0, "/opt/trn_rl_repo")

import ml_dtypes

P = 128  # partitions / tile edge
F = 128  # feature dim (OUT_F == IN_F == 128)
TF = 2 * F

BF16 = ml_dtypes.bfloat16


@dataclass
class Cfg:
    n_nodes: int = 50000
    n_cores: int = 8
    # mtab row split for the two gather passes; both segments must stay under
    # the 32768 int16 gather-index limit, and the boundary must align to a
    # phase-A chunk (ca*P rows) so lo gathers only depend on seg0 writes.
    half: int = 26624
    gw: int = 3  # windows per gather group
    ca: int = 16  # node-tiles per phase-A chunk
    sub: int = 12  # tiles per sub-gather (balances SWDGE queue desc-gen)

    @property
    def na(self) -> int:  # node tiles in M table
        return math.ceil(self.n_nodes / P)

    @property
    def npc(self) -> int:  # nodes per core
        assert self.n_nodes % self.n_cores == 0
        return self.n_nodes // self.n_cores

    @property
    def nwin(self) -> int:  # source windows per core
        return math.ceil(self.npc / P)


@dataclass
class Plan:
    """Static schedule shared by all cores + per-core host arrays."""

    T_lo: list  # tiles per (window, low-half), max over cores
    T_hi: list
    groups: list = field(default_factory=list)  # (ws, lo_tiles, hi_tiles)
    tiles_of: dict = field(default_factory=dict)  # (w, half) -> [tile ids]
    win_tiles: dict = field(default_factory=dict)  # w -> [(tile, half, pos_in_buf)]
    TT: int = 0
    # per-core packed arrays
    srel: list = field(default_factory=list)  # [P, TT] bf16
    gidx: list = field(default_factory=list)  # [P, 8*TT] int16


def _make_schedule(cfg: Cfg, T_lo, T_hi) -> Plan:
    plan = Plan(T_lo=T_lo, T_hi=T_hi)
    t = 0
    for g0 in range(0, cfg.nwin, cfg.gw):
        ws = list(range(g0, min(g0 + cfg.gw, cfg.nwin)))
        lo, hi = [], []
        for w in ws:
            for _ in range(T_lo[w]):
                plan.tiles_of.setdefault((w, 0), []).append(t)
                lo.append((w, t))
                t += 1
        for w in ws:
            for _ in range(T_hi[w]):
                plan.tiles_of.setdefault((w, 1), []).append(t)
                hi.append((w, t))
                t += 1
        plan.groups.append((ws, lo, hi))
    plan.TT = t
    # per window and half: list of (global tile id, position within the
    # group's lo/hi gather buffer) in consumption order
    for ws, lo, hi in plan.groups:
        for w in ws:
            plan.win_tiles[(w, 0)] = [
                (t2, pos) for pos, (w2, t2) in enumerate(lo) if w2 == w
            ]
            plan.win_tiles[(w, 1)] = [
                (t2, pos) for pos, (w2, t2) in enumerate(hi) if w2 == w
            ]
    return plan


def _plan(cfg: Cfg, edge_sources: np.ndarray, edge_targets: np.ndarray) -> Plan:
    src = edge_sources.astype(np.int64)
    tgt = edge_targets.astype(np.int64)
    npc, nwin = cfg.npc, cfg.nwin

    # bucket edges per (core, window, half)
    core = src // npc
    w_all = (src % npc) // P
    srel_all = (src % npc) % P
    hi_all = (tgt >= cfg.half).astype(np.int64)

    counts = np.zeros((cfg.n_cores, nwin, 2), np.int64)
    np.add.at(counts, (core, w_all, hi_all), 1)
    tmax = counts.max(axis=0)  # [nwin, 2]
    T_lo = [int(math.ceil(tmax[w, 0] / P)) for w in range(nwin)]
    T_hi = [int(math.ceil(tmax[w, 1] / P)) for w in range(nwin)]

    plan = _make_schedule(cfg, T_lo, T_hi)

    # pack per-core slot arrays
    order = np.lexsort((hi_all, w_all, core))
    src_s, w_s, srel_s, hi_s, tgt_s = (
        src[order],
        w_all[order],
        srel_all[order],
        hi_all[order],
        tgt[order],
    )
    bounds = {}
    keys = core[order] * (nwin * 2) + w_s * 2 + hi_s
    uniq, starts = np.unique(keys, return_index=True)
    starts = list(starts) + [len(keys)]
    for i, k in enumerate(uniq):
        bounds[int(k)] = (starts[i], starts[i + 1])

    for c in range(cfg.n_cores):
        srel_arr = np.full((plan.TT * P,), 255.0, np.float32)
        gidx_arr = np.zeros((plan.TT * P,), np.int16)
        for w in range(nwin):
            for h in (0, 1):
                k = c * (nwin * 2) + w * 2 + h
                if k not in bounds:
                    continue
                a, b = bounds[k]
                tiles = plan.tiles_of.get((w, h), [])
                assert (b - a) <= len(tiles) * P
                sr = srel_s[a:b]
                tg = tgt_s[a:b] - (cfg.half if h else 0)
                for i in range(b - a):
                    t = tiles[i // P]
                    j = i % P
                    s = t * P + j
                    srel_arr[s] = sr[i]
                    gidx_arr[s] = tg[i]
        srel_host = srel_arr.reshape(plan.TT, P).T.astype(BF16)  # [P, TT]
        g16 = gidx_arr.reshape(plan.TT * 8, 16).T  # [16, TT*8]
        gidx_host = np.tile(g16, (8, 1)).astype(np.int16)  # [P, TT*8]
        plan.srel.append(np.ascontiguousarray(srel_host))
        plan.gidx.append(np.ascontiguousarray(gidx_host))
    return plan


def _host_arrays(cfg: Cfg, inp: np.ndarray, W: np.ndarray):
    """Replicated input arrays: xT (transposed node features), wT, iota."""
    n = cfg.n_nodes
    xT = np.zeros((P, cfg.na * P), np.float32)
    xT[:, :n] = inp.T
    wT = np.ascontiguousarray(W.T)  # [F, 2F]
    iota = np.tile(np.arange(P, dtype=np.float32), (P, 1))
    return (
        np.ascontiguousarray(xT.astype(BF16)),
        np.ascontiguousarray(wT.astype(BF16)),
        np.ascontiguousarray(iota.astype(BF16)),
    )


def _xs_core(cfg: Cfg, inp: np.ndarray, c: int) -> np.ndarray:
    """Per-core input slice packed [P, nwin*F]: xs[p, w*F+f] = x[c*npc+w*P+p, f]."""
    npc, nwin = cfg.npc, cfg.nwin
    sl = np.zeros((nwin * P, F), np.float32)
    sl[:npc] = inp[c * npc : (c + 1) * npc]
    return np.ascontiguousarray(sl.reshape(nwin, P, F).transpose(1, 0, 2).reshape(P, nwin * F))


def _build(cfg: Cfg, plan: Plan, enable_asserts: bool = False):
    import concourse.bacc as bacc
    import concourse.tile as tile
    from concourse import mybir

    nc = bacc.Bacc(
        "TRN2",
        target_bir_lowering=False,
        debug=False,
        enable_asserts=enable_asserts,
        num_devices=cfg.n_cores,
        num_swdge_queues=4,
    )
    dt = mybir.dt

    na, nwin, npc = cfg.na, cfg.nwin, cfg.npc
    TT = plan.TT

    xT_d = nc.dram_tensor("xT", [P, na * P], dt.bfloat16, kind="ExternalInput")
    wT_d = nc.dram_tensor("wT", [P, TF], dt.bfloat16, kind="ExternalInput")
    iota_d = nc.dram_tensor("iota", [P, P], dt.bfloat16, kind="ExternalInput")
    xs_d = nc.dram_tensor("xs", [P, nwin * F], dt.float32, kind="ExternalInput")
    srel_d = nc.dram_tensor("srel", [P, TT], dt.bfloat16, kind="ExternalInput")
    gidx_d = nc.dram_tensor("gidx", [P, 8 * TT], dt.int16, kind="ExternalInput")
    y_d = nc.dram_tensor("y", [npc, F], dt.float32, kind="ExternalOutput")
    mtab_d = nc.dram_tensor("mtab", [na * P, F], dt.bfloat16, kind="Internal")

    lo_rows = min(cfg.half, na * P)

    max_tl = max((len(lo) for _, lo, _ in plan.groups), default=0)
    max_th = max((len(hi) for _, _, hi in plan.groups), default=0)
    max_tg = max((len(lo) + len(hi) for _, lo, hi in plan.groups), default=0)

    n_chunks = math.ceil(na / cfg.ca)

    from concourse.tile import add_dep_helper

    S = cfg.sub

    with tile.TileContext(nc) as tc:
        import contextlib

        with contextlib.ExitStack() as ctx:
            consts = ctx.enter_context(tc.tile_pool(name="consts", bufs=1))
            a_in = ctx.enter_context(tc.tile_pool(name="a_in", bufs=3))
            a_ps = ctx.enter_context(tc.tile_pool(name="a_ps", bufs=4, space="PSUM"))
            a_sg = ctx.enter_context(tc.tile_pool(name="a_sg", bufs=4))
            a_m = ctx.enter_context(tc.tile_pool(name="a_m", bufs=3))
            b_lo = ctx.enter_context(tc.tile_pool(name="b_lo", bufs=8))
            b_hi = ctx.enter_context(tc.tile_pool(name="b_hi", bufs=6))
            b_oh = ctx.enter_context(tc.tile_pool(name="b_oh", bufs=2))
            b_ps = ctx.enter_context(tc.tile_pool(name="b_ps", bufs=3, space="PSUM"))
            b_out = ctx.enter_context(tc.tile_pool(name="b_out", bufs=4))

            # ---- constants to SBUF ----
            # tiny consts on the SP queue (shared with mtab writes); bulky
            # consts + xt reads go on the Activation HWDGE queue so the first
            # matmul's inputs arrive immediately.
            wT_sb = consts.tile([P, TF], dt.bfloat16, tag="wT")
            nc.sync.dma_start(wT_sb[:], wT_d[:, :])
            iota_sb = consts.tile([P, P], dt.bfloat16, tag="iota")
            nc.sync.dma_start(iota_sb[:], iota_d[:, :])
            xs_sb = consts.tile([P, nwin * F], dt.float32, tag="xs")
            srel_sb = consts.tile([P, TT], dt.bfloat16, tag="srel")
            gidx_sb = consts.tile([P, 8 * TT], dt.int16, tag="gidx")
            # index consts ride the SP queue ahead of the mtab writes so the
            # first pass-1 gather isn't stuck behind compute-gated xt loads
            nc.sync.dma_start(gidx_sb[:], gidx_d[:, :])
            nc.sync.dma_start(srel_sb[:], srel_d[:, :])
            nc.sync.dma_start(xs_sb[:], xs_d[:, :])

            # ---- phase A: M table (2 node-tiles batched per PSUM bank) ----
            mdmas = []
            last_a_op = None
            for ci in range(n_chunks):
                c0 = ci * cfg.ca
                ca = min(cfg.ca, na - c0)
                xt = a_in.tile([P, cfg.ca * P], dt.bfloat16, tag="xt")
                nc.scalar.dma_start(xt[:, : ca * P], xT_d[:, c0 * P : (c0 + ca) * P])
                mtile = a_m.tile([P, cfg.ca * F], dt.bfloat16, tag="mtile")
                for k0 in range(0, ca, 2):
                    kk = min(2, ca - k0)
                    ps = a_ps.tile([P, 2 * TF], dt.float32, tag="psA")
                    for j in range(kk):
                        nc.tensor.matmul(
                            ps[:, j * TF : (j + 1) * TF],
                            lhsT=xt[:, (k0 + j) * P : (k0 + j + 1) * P],
                            rhs=wT_sb[:],
                            start=True,
                            stop=True,
                        )
                    psv = ps[:, : kk * TF].rearrange("p (k tf) -> p k tf", tf=TF)
                    sg = a_sg.tile([P, 2 * F], dt.float32, tag="sg")
                    sgv = sg[:, : kk * F].rearrange("p (k f) -> p k f", f=F)
                    nc.scalar.activation(
                        sgv, psv[:, :, 0:F], mybir.ActivationFunctionType.Sigmoid
                    )
                    mv = mtile[:, k0 * F : (k0 + kk) * F].rearrange(
                        "p (k f) -> p k f", f=F
                    )
                    last_a_op = nc.vector.tensor_tensor(
                        out=mv, in0=psv[:, :, F:TF], in1=sgv,
                        op=mybir.AluOpType.mult,
                    )
                out_ap = (
                    mtab_d[c0 * P : (c0 + ca) * P, :]
                    .rearrange("(k p) f -> p k f", p=P)
                )
                mdmas.append(
                    nc.sync.dma_start(
                        out_ap, mtile[:, : ca * F].rearrange("p (k f) -> p k f", f=F)
                    )
                )

            # ---- phase B: two passes (lo rows of mtab, then hi rows) ----
            # Pass 1 gathers depend only on seg0 mtab chunks, so their Q7
            # desc-gen + drains overlap the tail of phase A. Partial window
            # sums (lo contribution + input) park in SBUF until pass 2.
            seg_chunks = cfg.half // (cfg.ca * P)
            assert seg_chunks * cfg.ca * P == cfg.half
            mdma_seg = {0: mdmas[:seg_chunks], 1: mdmas[seg_chunks:]}
            part = consts.tile([P, nwin * F], dt.float32, tag="part")

            qi = [0]  # strict round-robin SWDGE queue counter
            first_iseq = [True]

            def emit_gathers(h, seq):
                """Emit sub-gathers for one group's lo or hi tile run."""
                subs = {}
                if not seq:
                    return subs
                t0, n = seq[0][1], len(seq)
                for j0 in range(0, n, S):
                    ln = min(S, n - j0)
                    pool = b_hi if h else b_lo
                    buf = pool.tile(
                        [P, S * F], dt.bfloat16, tag="hib" if h else "lob"
                    )
                    subs[j0] = buf
                    in_ap = (
                        mtab_d[cfg.half : na * P, :] if h else mtab_d[0:lo_rows, :]
                    )
                    g = nc.gpsimd.dma_gather(
                        out_ap=buf[:, : ln * F].rearrange("p (t e) -> p t e", e=F),
                        in_ap=in_ap,
                        idxs_ap=gidx_sb[:, 8 * (t0 + j0) : 8 * (t0 + j0 + ln)],
                        num_idxs=ln * P,
                        num_idxs_reg=ln * P,
                        elem_size=F,
                        single_packet=False,
                        queue_num=qi[0] % 4,
                    )
                    qi[0] += 1
                    for m in mdma_seg[h]:
                        add_dep_helper(g.ins, m.ins, reason="mtab RAW")
                return subs

            def emit_onehot(seq):
                if not seq:
                    return None, 0
                tg0, ntg = seq[0][1], len(seq)
                oh = b_oh.tile([P, max(max_tg, 1) * P], dt.bfloat16, tag="oh")
                iseq = nc.vector.tensor_tensor(
                    out=oh[:, : ntg * P].rearrange("p (t e) -> p t e", e=P),
                    in0=srel_sb[:, tg0 : tg0 + ntg]
                    .unsqueeze(2)
                    .to_broadcast([P, ntg, P]),
                    in1=iota_sb[:].unsqueeze(1).to_broadcast([P, ntg, P]),
                    op=mybir.AluOpType.is_equal,
                )
                if first_iseq[0] and last_a_op is not None:
                    # keep DVE queue ordered: phase-A muls before one-hots
                    add_dep_helper(iseq.ins, last_a_op.ins, reason="DVE order")
                    first_iseq[0] = False
                return oh, tg0

            def accumulate(w, h, oh, tg0, subs, out_sl, add_sl):
                """ps = sum of window tiles; out_sl = ps + add_sl (or copy)."""
                tiles = plan.win_tiles.get((w, h), [])
                if not tiles:
                    nc.vector.tensor_copy(out_sl, add_sl)
                    return
                ps = b_ps.tile([P, F], dt.float32, tag="psB")
                for i, (t, pos) in enumerate(tiles):
                    buf = subs[(pos // S) * S]
                    col = pos % S
                    nc.tensor.matmul(
                        ps[:],
                        lhsT=oh[:, (t - tg0) * P : (t - tg0 + 1) * P],
                        rhs=buf[:, col * F : (col + 1) * F],
                        start=(i == 0),
                        stop=(i == len(tiles) - 1),
                    )
                nc.vector.tensor_add(out_sl, ps[:], add_sl)

            # pass 1: lo rows -> part[w] = sum_lo + xs[w]
            for ws, lo, hi in plan.groups:
                subs = emit_gathers(0, lo)
                oh, tg0 = emit_onehot(lo)
                for w in ws:
                    accumulate(
                        w, 0, oh, tg0, subs,
                        part[:, w * F : (w + 1) * F],
                        xs_sb[:, w * F : (w + 1) * F],
                    )

            # pass 2: hi rows -> y[w] = sum_hi + part[w]
            for ws, lo, hi in plan.groups:
                subs = emit_gathers(1, hi)
                oh, tg0 = emit_onehot(hi)
                for w in ws:
                    rows = min(P, npc - w * P)
                    ot = b_out.tile([P, F], dt.float32, tag="ot")
                    accumulate(
                        w, 1, oh, tg0, subs, ot[:], part[:, w * F : (w + 1) * F]
                    )
                    nc.sync.dma_start(y_d[w * P : w * P + rows, :], ot[:rows, :])

    nc.compile()
    return nc


def _in_maps(cfg: Cfg, plan: Plan, inp: np.ndarray, W: np.ndarray):
    xT, wT, iota = _host_arrays(cfg, inp, W)
    maps = []
    for c in range(cfg.n_cores):
        maps.append(
            {
                "xT": xT,
                "wT": wT,
                "iota": iota,
                "xs": _xs_core(cfg, inp, c),
                "srel": plan.srel[c],
                "gidx": plan.gidx[c],
            }
        )
    return maps


def _install_ntff_hook():
    """Provide the antenv.axon_hooks shim trn_boot expects, so trace=True
    can capture NTFF profiles. Silently degrades if anything is missing."""
    try:
        import antenv.axon_hooks  # noqa: F401

        return
    except ImportError:
        pass
    try:
        import types

        import antenv

        mod = types.ModuleType("antenv.axon_hooks")
        _hook = [None]
        mod.set_axon_ntff_profile_hook = lambda h: _hook.__setitem__(0, h)
        mod.get_axon_ntff_profile_hook = lambda: _hook[0]
        sys.modules["antenv.axon_hooks"] = mod
        antenv.axon_hooks = mod
        from trn_agent_boot import trn_boot

        mod.set_axon_ntff_profile_hook(
            trn_boot._ntff_profile_via_ctypes("/opt/axon/libaxon_pjrt.so")
        )
    except Exception:
        pass


def kernel(**inputs) -> np.ndarray:
    inp = np.asarray(inputs["input"], np.float32)
    W = np.asarray(inputs["W"], np.float32)
    es = np.asarray(inputs["edge_sources"]).astype(np.int64)
    et = np.asarray(inputs["edge_targets"]).astype(np.int64)

    cfg = Cfg(n_nodes=inp.shape[0])
    plan = _plan(cfg, es, et)
    nc = _build(cfg, plan)

    from concourse.bass_utils import run_bass_kernel_spmd

    if bool(int(os.environ.get("GGC_TRACE", "0"))):
        _install_ntff_hook()
    res = run_bass_kernel_spmd(
        nc,
        _in_maps(cfg, plan, inp, W),
        core_ids=list(range(cfg.n_cores)),
        trace=bool(int(os.environ.get("GGC_TRACE", "0"))),
    )
    out = np.concatenate([res.results[c]["y"] for c in range(cfg.n_cores)], axis=0)
    if bool(int(os.environ.get("GGC_TRACE", "0"))):
        kernel.last_results = res  # stash for test harness
    return out

